# revision 1
# baseline (speedup 1.0000x reference)
"""Trainium2 Bass kernel for nn_E74Layer (delta-rule recurrent layer).

Strategy:
  - Host: fuse W_in into the four cell projections (h is only an
    intermediate): k|v|q|z = x @ (W_* @ W_in)^T.  8x device-FLOP cut.
  - Device (8 cores, data-parallel over batch B=8, one batch row each):
    chunked delta-rule with chunk C=128 over T=2048:
      * per-chunk: PE-transpose x, project to k,v,q,z (float32r fast path),
        normalize k, G = K K^T, triangular solve (I+L)W=[K|V] by nilpotent
        Neumann doubling (5 iterations suffice: |L^32| ~ 1e-12),
        chunk summary matrices C1T/C2n/P2T (all layout-natural matmuls),
      * tiny serial 64x64 affine chain over the 16 chunk states,
      * parallel readout R, y = tanh(R)*sigmoid(Z), out = y @ W_out^T.
"""
import numpy as np

T, B, DIM, D_INNER, N_STATE = 2048, 8, 1024, 2048, 64
C = 128                 # chunk size (tokens)
NCH = T // C            # 16 chunks
NS = N_STATE            # 64
SOLVE_ITERS = 5         # Sum_{j<32} (-L)^j; |L^32| ~ 1e-12 for this data



# copy-engine assignment per site: 'v'=DVE 'a'=ACT 'd'=DMA(SP) 'A'=DMA(ACT)
# 'g'=DMA(gpsimd); pool sizes
CFG = {
    'xt0': 'v', 'xt1': 'a', 'kvqz': 'v', 'kt': 'v', 'qt': 'a', 'zt': 'a',
    'm0': 'v', 'm1': 'a', 'mt0': 'a', 'mt1': 'v', 'hc1': 'v',
    'out0': 'v', 'out1': 'a',
    'xin_bufs': 4, 'solve_bufs': 4, 'hh_bufs': 6,
    'psbig_bufs': 2, 'pssolve_bufs': 3, 'pssmall_bufs': 3, 'psB_bufs': 0,
    'mt_transpose': False, 'gp_masks': False, 'g0': 'a',
    'bf_corr': False, 'gp_twin': False,
}

def _build_nc(reps=1, loop_n=None):
    import concourse.bass as bass
    import concourse.mybir as mybir
    from concourse import bacc
    from concourse.tile import TileContext

    dt = mybir.dt
    nc = bacc.Bacc(None, target_bir_lowering=False)

    x_rows = 2 * C if loop_n else T
    x_d = nc.declare_dram_parameter("x", [x_rows, DIM], dt.float32, isOutput=False)
    wft_d = nc.declare_dram_parameter("wft", [DIM, 4 * NS], dt.float32r, isOutput=False)
    wot_d = nc.declare_dram_parameter("wot", [NS, DIM], dt.float32r, isOutput=False)
    ident_d = nc.declare_dram_parameter("ident", [128, 128], dt.float32, isOutput=False)
    mask_ln_d = nc.declare_dram_parameter("mask_ln", [C, C], dt.float32, isOutput=False)
    mask_un_d = nc.declare_dram_parameter("mask_un", [C, C], dt.float32, isOutput=False)
    mask_ui_d = nc.declare_dram_parameter("mask_ui", [C, C], dt.float32, isOutput=False)
    out_d = nc.declare_dram_parameter("out", [x_rows, DIM], dt.float32, isOutput=True)

    f32 = dt.float32
    f32r = dt.float32r
    cfg = CFG

    def copyto(which, out_ap, in_ap):
        eng = cfg[which]
        if eng == 'v':
            nc.vector.tensor_copy(out_ap, in_ap)
        elif eng == 'a':
            nc.scalar.copy(out_ap, in_ap)
        elif eng == 'd':
            nc.sync.dma_start(out=out_ap, in_=in_ap)
        elif eng == 'A':
            nc.scalar.dma_start(out=out_ap, in_=in_ap)
        elif eng == 'g':
            nc.gpsimd.dma_start(out=out_ap, in_=in_ap)
        else:
            raise ValueError(eng)

    from contextlib import ExitStack
    with TileContext(nc) as tc:
        with ExitStack() as _es:
            constp = _es.enter_context(tc.tile_pool(name="const", bufs=1))
            xinp = _es.enter_context(tc.tile_pool(name="xin", bufs=cfg['xin_bufs']))
            xtrp = _es.enter_context(tc.tile_pool(name="xtr", bufs=3))
            kvqzp = _es.enter_context(tc.tile_pool(name="kvqz", bufs=3))
            workp = _es.enter_context(tc.tile_pool(name="work", bufs=4))
            solvep = _es.enter_context(tc.tile_pool(name="solve", bufs=cfg['solve_bufs']))
            wkeepp = _es.enter_context(tc.tile_pool(name="wkeep", bufs=NCH + 1))
            m1keepp = _es.enter_context(tc.tile_pool(name="m1keep", bufs=NCH + 1))
            p2keepp = _es.enter_context(tc.tile_pool(name="p2keep", bufs=NCH + 1))
            ztkeepp = _es.enter_context(tc.tile_pool(name="ztkeep", bufs=NCH + 1))
            hhp = _es.enter_context(tc.tile_pool(name="hh", bufs=cfg['hh_bufs']))
            yp = _es.enter_context(tc.tile_pool(name="Y", bufs=NCH + 2))
            outpool = _es.enter_context(tc.tile_pool(name="outp", bufs=4))
            psbig = _es.enter_context(tc.tile_pool(
                name="psbig", bufs=cfg['psbig_bufs'], space="PSUM"))
            pssolve = _es.enter_context(tc.tile_pool(
                name="pssolve", bufs=cfg['pssolve_bufs'], space="PSUM"))
            pssmall = _es.enter_context(tc.tile_pool(
                name="pssmall", bufs=cfg['pssmall_bufs'], space="PSUM"))
            if cfg['psB_bufs'] > 0:
                psB = _es.enter_context(tc.tile_pool(
                    name="psB", bufs=cfg['psB_bufs'], space="PSUM"))
                psrt_pool, psrt_tag = psB, "rt"
                pso_pool, pso_tag = psB, "rt"
            else:
                psrt_pool, psrt_tag = pssmall, "small"
                pso_pool, pso_tag = psbig, "big"

            # constants
            wft = constp.tile([128, 8 * 4 * NS], f32r, tag="wft")
            nc.sync.dma_start(
                out=wft[:].rearrange("p (s n) -> p s n", s=8),
                in_=wft_d.ap().rearrange("(s p) n -> p s n", p=128))
            wot = constp.tile([NS, DIM], f32r, tag="wot")
            nc.sync.dma_start(out=wot[:], in_=wot_d[:])
            ident = constp.tile([128, 128], f32, tag="ident")
            nc.sync.dma_start(out=ident[:], in_=ident_d[:])
            mask_ln = constp.tile([C, C], f32, tag="mask_ln")
            nc.sync.dma_start(out=mask_ln[:], in_=mask_ln_d[:])
            mask_un = constp.tile([C, C], f32, tag="mask_un")
            nc.sync.dma_start(out=mask_un[:], in_=mask_un_d[:])
            mask_ui = constp.tile([C, C], f32, tag="mask_ui")
            nc.sync.dma_start(out=mask_ui[:], in_=mask_ui_d[:])

            # reps>1: repeat computation for timing differentials
            import contextlib
            loop_cm = (tc.For_i(0, loop_n, 1) if loop_n
                       else contextlib.nullcontext())
            with loop_cm:
              for _rep in range(reps):
                Y0 = yp.tile([NS, NS], f32, tag="Y")
                nc.gpsimd.memset(Y0[:], 0.0)
                y_tiles = [Y0]

                chunk_keep = []

                # ---------------- phase A + chain ----------------
                for c in range(NCH):
                    if c % 2 == 0:
                        xt2 = xinp.tile([128, 2 * DIM], f32, tag="x")
                        if loop_n:
                            xsrc = x_d.ap()
                        else:
                            xsrc = x_d[2 * (c // 2) * C:
                                       (2 * (c // 2) + 2) * C, :]
                        nc.sync.dma_start(
                            out=xt2[:].rearrange("p (j d) -> p j d", j=2),
                            in_=xsrc.rearrange("(j p) d -> p j d", p=128))
                    xt = xt2[:, (c % 2) * DIM:(c % 2 + 1) * DIM]

                    # transpose x chunk: 8x [128,128] -> xTr (f32r)
                    xTr = xtrp.tile([128, DIM], f32r, tag="xtr")
                    for half in range(2):
                        pst = psbig.tile([128, 512], f32, tag="big")
                        for j in range(4):
                            s = 4 * half + j
                            nc.tensor.transpose(
                                pst[:, 128 * j:128 * (j + 1)],
                                xt[:, 128 * s:128 * (s + 1)], ident[:])
                        copyto('xt%d' % half,
                               xTr[:, 512 * half:512 * (half + 1)], pst[:])

                    # kvqz = x @ Wf^T  [128 tok, 256]
                    psk = psbig.tile([128, 4 * NS], f32, tag="big")
                    for s in range(8):
                        nc.tensor.matmul(psk[:], xTr[:, 128 * s:128 * (s + 1)],
                                         wft[:, 256 * s:256 * (s + 1)],
                                         start=(s == 0), stop=(s == 7))
                    kvqz = kvqzp.tile([128, 4 * NS], f32, tag="kvqz")
                    copyto('kvqz', kvqz[:], psk[:])

                    # normalize k rows (in place)
                    ksq = workp.tile([128, NS], f32, tag="ksq")
                    ss = workp.tile([128, 1], f32, tag="ss")
                    nc.scalar.activation(ksq[:], kvqz[:, 0:NS],
                                         mybir.ActivationFunctionType.Square,
                                         accum_out=ss[:])
                    nrm = workp.tile([128, 1], f32, tag="nrm")
                    nc.scalar.activation(nrm[:], ss[:],
                                         mybir.ActivationFunctionType.Sqrt,
                                         bias=0.0)
                    nc.vector.tensor_scalar_add(nrm[:], nrm[:], 1e-6)
                    rnrm = workp.tile([128, 1], f32, tag="rnrm")
                    nc.vector.reciprocal(rnrm[:], nrm[:])
                    nc.vector.tensor_scalar_mul(kvqz[:, 0:NS], kvqz[:, 0:NS],
                                                rnrm[:])

                    # transposes KT, QT, ZT
                    KT = workp.tile([NS, C], f32, tag="KT")
                    QT = workp.tile([NS, C], f32, tag="QT")
                    ZT = ztkeepp.tile([NS, C], f32, tag="ZT")
                    pskt = pssmall.tile([NS, C], f32, tag="small")
                    nc.tensor.transpose(pskt[:], kvqz[:, 0:NS], ident[:])
                    copyto('kt', KT[:], pskt[:])
                    psqt = pssmall.tile([NS, C], f32, tag="small")
                    nc.tensor.transpose(psqt[:], kvqz[:, 2 * NS:3 * NS], ident[:])
                    copyto('qt', QT[:], psqt[:])
                    pszt = pssmall.tile([NS, C], f32, tag="small")
                    nc.tensor.transpose(pszt[:], kvqz[:, 3 * NS:4 * NS], ident[:])
                    copyto('zt', ZT[:], pszt[:])

                    # G and masks -> M0 (-L), M0T (-L^T)
                    psg = pssolve.tile([C, C], f32, tag="solve")
                    nc.tensor.matmul(psg[:], KT[:], KT[:], start=True, stop=True)
                    M0 = solvep.tile([C, C], f32, tag="M")
                    M0T = solvep.tile([C, C], f32, tag="MT")
                    if cfg['gp_masks']:
                        Gsb = solvep.tile([C, C], f32, tag="Gsb")
                        copyto('g0', Gsb[:], psg[:])
                        nc.gpsimd.tensor_mul(M0[:], Gsb[:], mask_ln[:])
                        nc.gpsimd.tensor_mul(M0T[:], Gsb[:], mask_un[:])
                    else:
                        nc.vector.tensor_mul(M0[:], psg[:], mask_ln[:])
                        nc.vector.tensor_mul(M0T[:], psg[:], mask_un[:])

                    # M1T = mask_ui * (K Q^T)
                    psm1 = pssolve.tile([C, C], f32, tag="solve")
                    nc.tensor.matmul(psm1[:], KT[:], QT[:], start=True, stop=True)
                    M1T = m1keepp.tile([C, C], f32, tag="M1T")
                    nc.vector.tensor_mul(M1T[:], psm1[:], mask_ui[:])

                    # solve (I+L) W = [K|V]: W <- W + M_k W; M_{k+1} = M_k^2;
                    # M_{k+1}^T via PE transpose
                    # 4 doubling iters (j<16) + bf16 L^8(L^8 W) correction
                    bf16 = dt.bfloat16
                    wc_ap = kvqz[:, 0:2 * NS]
                    Mk, MkT = M0, M0T
                    M3bf = None
                    n_it = 4 if cfg['bf_corr'] else 5
                    for it in range(n_it):
                        psw = pssolve.tile([C, C], f32, tag="solve")
                        nc.tensor.matmul(psw[:], MkT[:], wc_ap,
                                         start=True, stop=True)
                        if it == n_it - 1 and not cfg['bf_corr']:
                            Wn = wkeepp.tile([C, 2 * NS], f32, tag="W")
                        else:
                            Wn = solvep.tile([C, 2 * NS], f32, tag="Wtmp")
                        nc.vector.tensor_add(Wn[:], psw[:, 0:2 * NS], wc_ap)
                        wc_ap = Wn[:]
                        if it == 3 and cfg['bf_corr']:
                            # bf16 twin of W4 for the correction
                            W4bf = solvep.tile([C, 2 * NS], bf16, tag="Wbf")
                            if cfg['gp_twin']:
                                nc.gpsimd.tensor_copy(W4bf[:], Wn[:])
                            else:
                                nc.vector.tensor_copy(W4bf[:], Wn[:])  # DVE
                        if it < n_it - 1:
                            psm2 = pssolve.tile([C, C], f32, tag="solve")
                            nc.tensor.matmul(psm2[:], MkT[:], Mk[:],
                                             start=True, stop=True)
                            Mn = solvep.tile([C, C], f32, tag="M")
                            copyto('m%d' % (it % 2), Mn[:], psm2[:])
                            MnT = solvep.tile([C, C], f32, tag="MT")
                            psmt = pssolve.tile([C, C], f32, tag="solve")
                            nc.tensor.matmul(psmt[:], Mk[:], MkT[:],
                                             start=True, stop=True)
                            copyto('mt%d' % (it % 2), MnT[:], psmt[:])
                            if it == 2 and cfg['bf_corr']:
                                # bf16 twin of M3T (= (L^8)^T) for correction
                                M3bf = solvep.tile([C, C], bf16, tag="M3bf")
                                if cfg['gp_twin']:
                                    nc.gpsimd.tensor_copy(M3bf[:], MnT[:])
                                else:
                                    nc.vector.tensor_copy(M3bf[:], MnT[:])
                            Mk, MkT = Mn, MnT
                    if cfg['bf_corr']:
                        # correction: W5 = W4 + M3 (M3 W4) in bf16
                        pst1 = pssolve.tile([C, C], f32, tag="solve")
                        nc.tensor.matmul(pst1[:], M3bf[:], W4bf[:],
                                         start=True, stop=True)
                        tmpbf = solvep.tile([C, 2 * NS], bf16, tag="Wbf")
                        nc.vector.tensor_copy(tmpbf[:], pst1[:, 0:2 * NS])
                        pst2 = pssolve.tile([C, C], f32, tag="solve")
                        nc.tensor.matmul(pst2[:], M3bf[:], tmpbf[:],
                                         start=True, stop=True)
                        W5 = wkeepp.tile([C, 2 * NS], f32, tag="W")
                        nc.vector.tensor_add(W5[:], pst2[:, 0:2 * NS], wc_ap)
                    else:
                        W5 = Wn
                    # W5: B_w = cols 0:64, A_w = cols 64:128

                    # chain coefficients: PT = I - C2' (=I - B_w^T K),
                    # C1T = K^T A_w  — both [64,64] at base partition 0
                    PT = hhp.tile([NS, NS], f32, tag="PT")
                    C1T = hhp.tile([NS, NS], f32, tag="C1T")
                    psc2 = pssmall.tile([NS, C], f32, tag="small")
                    nc.tensor.matmul(psc2[:, 0:NS], W5[:, 0:NS], kvqz[:, 0:NS],
                                     start=True, stop=True)
                    nc.vector.tensor_sub(PT[:], ident[0:NS, 0:NS],
                                         psc2[:, 0:NS])
                    psc1 = pssmall.tile([NS, C], f32, tag="small")
                    nc.tensor.matmul(psc1[:, 0:NS], kvqz[:, 0:NS],
                                     W5[:, NS:2 * NS], start=True, stop=True)
                    copyto('hc1', C1T[:], psc1[:, 0:NS])

                    # P2T = Q^T - B_w^T M1T
                    psp2 = pssmall.tile([NS, C], f32, tag="small")
                    nc.tensor.matmul(psp2[:], W5[:, 0:NS], M1T[:],
                                     start=True, stop=True)
                    P2T = p2keepp.tile([NS, C], f32, tag="P2T")
                    nc.vector.tensor_sub(P2T[:], QT[:], psp2[:])

                    chunk_keep.append((W5, M1T, P2T, ZT))

                    # serial chain step: Y_{c+1} = H_c Y_c  (one matmul)
                    with tc.high_priority():
                        psy = pssmall.tile([NS, C], f32, tag="small")
                        nc.tensor.matmul(psy[:, 0:NS], PT[:], y_tiles[c][:],
                                         start=True, stop=False)
                        nc.tensor.matmul(psy[:, 0:NS], ident[0:NS, 0:NS],
                                         C1T[:], start=False, stop=True)
                        Yn = yp.tile([NS, NS], f32, tag="Y")
                        nc.vector.tensor_copy(Yn[:], psy[:, 0:NS])
                        y_tiles.append(Yn)


                # ---------------- phase B: readout (dedicated psum pool) ----
                for c in range(NCH):
                    W5, M1T, P2T, ZT = chunk_keep[c]
                    psrt = psrt_pool.tile([NS, C], f32, tag=psrt_tag)
                    nc.tensor.matmul(psrt[:], W5[:, NS:2 * NS], M1T[:],
                                     start=True, stop=False)
                    nc.tensor.matmul(psrt[:], y_tiles[c][:], P2T[:],
                                     start=False, stop=True)
                    th = workp.tile([NS, C], f32, tag="th")
                    nc.scalar.activation(th[:], psrt[:],
                                         mybir.ActivationFunctionType.Tanh)
                    sg = workp.tile([NS, C], f32, tag="sg")
                    nc.scalar.activation(sg[:], ZT[:],
                                         mybir.ActivationFunctionType.Sigmoid)
                    yT = workp.tile([NS, C], f32r, tag="yT")
                    nc.vector.tensor_mul(yT[:], th[:], sg[:])

                    if c % 2 == 0:
                        out_sb2 = outpool.tile([128, 2 * DIM], f32, tag="out")
                    off = (c % 2) * DIM
                    for half in range(2):
                        pso = pso_pool.tile([128, 512], f32, tag=pso_tag)
                        nc.tensor.matmul(pso[:], yT[:],
                                         wot[:, 512 * half:512 * (half + 1)],
                                         start=True, stop=True)
                        copyto('out%d' % half,
                               out_sb2[:, off + 512 * half:
                                       off + 512 * (half + 1)], pso[:])
                    if c % 2 == 1:
                        m = c // 2
                        odst = (out_d.ap() if loop_n
                                else out_d[2 * m * C:(2 * m + 2) * C, :])
                        nc.sync.dma_start(
                            out=odst.rearrange("(j p) d -> p j d", p=128),
                            in_=out_sb2[:].rearrange("p (j d) -> p j d", j=2))

    nc.finalize()
    return nc


_NC_CACHE = {}


def _get_runner(reps=1, loop_n=None):
    """Build nc + a cached jitted SPMD callable (jit traced once)."""
    key = ("runner", reps, loop_n)
    if key in _NC_CACHE:
        return _NC_CACHE[key]
    import jax
    import numpy as _np
    from jax.sharding import Mesh, PartitionSpec
    from jax.experimental.shard_map import shard_map
    from concourse import bass2jax
    from concourse import mybir

    nc = _build_nc(reps=reps, loop_n=loop_n)
    bass2jax.install_neuronx_cc_hook()

    in_names = []
    out_names = []
    out_avals = []
    zero_outs = []
    partition_name = (nc.partition_id_tensor.name
                      if nc.partition_id_tensor else None)
    for alloc in nc.m.functions[0].allocations:
        if not isinstance(alloc, mybir.MemoryLocationSet):
            continue
        name = alloc.memorylocations[0].name
        if alloc.kind == "ExternalInput":
            if name != partition_name:
                in_names.append(name)
        elif alloc.kind == "ExternalOutput":
            out_names.append(name)
            shape = tuple(alloc.tensor_shape)
            dtype = mybir.dt.np(alloc.dtype)
            out_avals.append(jax.core.ShapedArray(shape, dtype))
            zero_outs.append(_np.zeros(shape, dtype))
    n_params = len(in_names)
    n_outs = len(out_avals)
    all_in_names = in_names + out_names
    if partition_name is not None:
        all_in_names = all_in_names + [partition_name]

    def _body(*args):
        operands = list(args)
        if partition_name is not None:
            operands.append(bass2jax.partition_id_tensor())
        outs = bass2jax._bass_exec_p.bind(
            *operands,
            out_avals=tuple(out_avals),
            in_names=tuple(all_in_names),
            out_names=tuple(out_names),
            lowering_input_output_aliases=(),
            sim_require_finite=True,
            sim_require_nnan=True,
            nc=nc,
        )
        return tuple(outs)

    devices = jax.devices()[:B]
    mesh = Mesh(_np.asarray(devices), ("core",))
    in_specs = (PartitionSpec("core"),) * (n_params + n_outs)
    out_specs = (PartitionSpec("core"),) * n_outs
    sharded = jax.jit(
        shard_map(_body, mesh=mesh, in_specs=in_specs, out_specs=out_specs,
                  check_rep=False),
        keep_unused=True,
    )

    def run(in_maps):
        per_core = [[_np.asarray(m[nm]) for nm in in_names] for m in in_maps]
        concat_in = [
            _np.concatenate([per_core[c][i] for c in range(B)], axis=0)
            for i in range(n_params)
        ]
        concat_zero = [
            _np.concatenate([z] * B, axis=0) for z in zero_outs
        ]
        outs = sharded(*concat_in, *concat_zero)
        outs = [_np.asarray(o) for o in outs]
        result = []
        for c in range(B):
            m = {}
            for i, nm in enumerate(out_names):
                rows = zero_outs[i].shape[0]
                m[nm] = outs[i][c * rows:(c + 1) * rows]
            result.append(m)
        return result

    _NC_CACHE[key] = (run, sharded)
    return _NC_CACHE[key]


def kernel(x, W_in, W_k, W_v, W_q, W_z, W_out):
    x = np.asarray(x, dtype=np.float32)
    # host-side weight fusion (fp64 for exactness)
    Wkvqz = np.concatenate([np.asarray(W_k), np.asarray(W_v),
                            np.asarray(W_q), np.asarray(W_z)], axis=0)
    Wf = Wkvqz.astype(np.float64) @ np.asarray(W_in).astype(np.float64)
    WfT = np.ascontiguousarray(Wf.T).astype(np.float32)        # [DIM, 256]
    W_outT = np.ascontiguousarray(np.asarray(W_out).T).astype(np.float32)

    ident = np.eye(128, dtype=np.float32)
    mask_ln = -np.tril(np.ones((C, C), np.float32), -1)
    mask_un = -np.triu(np.ones((C, C), np.float32), 1)
    mask_ui = np.triu(np.ones((C, C), np.float32), 0)

    run, _ = _get_runner(reps=1)
    in_maps = []
    for b in range(B):
        in_maps.append({
            "x": np.ascontiguousarray(x[:, b, :]),
            "wft": WfT, "wot": W_outT, "ident": ident,
            "mask_ln": mask_ln, "mask_un": mask_un, "mask_ui": mask_ui,
        })
    results = run(in_maps)
    out = np.stack([results[b]["out"] for b in range(B)], axis=1)
    return out.astype(np.float32)

